# revision 22
# baseline (speedup 1.0000x reference)
"""DeepSeek layer (MLA attention + shared/routed MoE) on 8 TRN2 NeuronCores.

Data-parallel over tokens: core c handles batch c//4, tokens [(c%4)*256, ...).
Activations live feature-major [feature, token] on device; host pre-transposes
weights (bf16) and precomputes the first rmsnorm (depends only on input X).
Router logits are computed in fp32 so top-2 expert selection matches the
reference; expert matmuls run in bf16.
"""

import numpy as np
import ml_dtypes

import concourse.bass as bass
import concourse.tile as tile
from concourse import bacc, mybir
from concourse.bass_utils import run_bass_kernel_spmd
from concourse.masks import make_identity

BF16 = mybir.dt.bfloat16
F32 = mybir.dt.float32
FP8 = mybir.dt.float8e4
DR = mybir.MatmulPerfMode.DoubleRow
AX = mybir.AxisListType.X
ALU = mybir.AluOpType
ACTF = mybir.ActivationFunctionType

# fp8 scale factors: weights are ~N(0, 0.02) and the FFN intermediate is
# ~0.1-scale; both sit in fp8e4's subnormal range unscaled. Scales are
# folded back out on the PSUM->SBUF path.
WS = 64.0            # weight scale for all fp8 weight matrices
HS = 16.0            # scale for the swiglu intermediate hp

P = 128
D = 1024
KD = D // P          # 8 feature chunks
S = 1024             # keys per batch
TQ = 256             # query tokens per core
H = 4
DC = 256             # compressed kv dim == dk
F = 1024
KF = F // P
E = 8
EPS = 1e-6
SCALE = 1.0 / 16.0   # 1/sqrt(dk)

_CACHE = {}


def _r(ap, n=None):
    """Host-permuted DRAM [P, C*N] -> [P, C, N] view (contiguous)."""
    c = ap.shape[-1]
    n = n if n is not None else c // KD
    return ap.rearrange("p (k n) -> p k n", n=n)


def build_program():
    nc = bacc.Bacc(None)

    def dma_k(dst, src_ap, parts):
        """Issue a [P, K, N] DMA as `parts` k-chunk DMAs so the transfer
        spreads across DMA queues instead of serializing on one."""
        kk = dst.shape[-2]
        step = kk // parts
        for i in range(0, kk, step):
            nc.sync.dma_start(dst[:, i:i + step, :], src_ap[:, i:i + step, :])

    # All tensors are host-permuted to partition-major [P, chunks*N] so each
    # DMA is one contiguous segment per partition (descriptor-rate matters).
    din = {}
    for name, shape, dt in [
        ("nxt", [P, KD * S], FP8),
        ("nxq", [P, KD * TQ], FP8),
        ("xt", [P, KD * TQ], F32),
        ("wq", [P, KD * D], FP8),
        ("wkc", [P, KD * DC], FP8),
        ("wvc", [P, KD * DC], FP8),
        ("wo", [P, KD * D], FP8),
        ("wr", [P, KD * E], F32),
        ("ebias", [1, E], F32),
        ("sw1", [P, KD * F], FP8),
        ("sw3", [P, KD * F], FP8),
        ("sw2", [P, KF * D], FP8),
        ("ew1", [E, P, KD * F], FP8),
        ("ew3", [E, P, KD * F], FP8),
        ("ew2", [E, P, KF * D], FP8),
    ]:
        din[name] = nc.dram_tensor(name, shape, dt, kind="ExternalInput")
    outt = nc.dram_tensor("outt", [P, KD * TQ], F32, kind="ExternalOutput")

    with tile.TileContext(nc) as tc:
        with (
            tc.tile_pool(name="const", bufs=1) as const,
            tc.tile_pool(name="persist", bufs=1) as persist,
        ):
            ones_bf = const.tile([P, 1], BF16)
            nc.vector.memset(ones_bf, 1.0)
            ones_cf = const.tile([P, 1], F32)
            nc.vector.memset(ones_cf, 1.0)
            ones_row = const.tile([1, P], F32)
            nc.vector.memset(ones_row, 1.0)
            eps1 = const.tile([1, 1], F32)
            nc.vector.memset(eps1, EPS)
            ident = const.tile([P, P], F32)
            make_identity(nc, ident)
            ebias_b = const.tile([P, E], F32)
            nc.sync.dma_start(ebias_b, din["ebias"][:].to_broadcast([P, E]))

            xpT = persist.tile([P, KD, TQ], F32)      # X' = X + attn out
            nx2 = persist.tile([P, KD, TQ], FP8)      # rmsnorm2(X') fp8
            cb = persist.tile([P, E, TQ], F32)        # combine weights bcast
            outT = persist.tile([P, KD, TQ], F32)

            # shared-expert weights, prefetched during attention so the MoE
            # phase starts without a DMA stall (DMAs issued after the
            # attention inputs below)
            w1_0 = persist.tile([P, KD, F], FP8)
            w3_0 = persist.tile([P, KD, F], FP8)
            w2_0 = persist.tile([P, KF, D], FP8)

            # ---------------- attention ----------------
            with (
                tc.tile_pool(name="ainp", bufs=1) as ainp,
                tc.tile_pool(name="awts", bufs=1) as awts,
                tc.tile_pool(name="wostr", bufs=2) as wostr,
                tc.tile_pool(name="aact", bufs=1) as aact,
                tc.tile_pool(name="psA", bufs=3, space="PSUM") as psA,
                tc.tile_pool(name="psD", bufs=4, space="PSUM") as psD,
            ):
                # DMA issue order: Q-path operands first so PE starts early
                nxq = ainp.tile([P, KD, TQ], FP8)
                nc.sync.dma_start(nxq, _r(din["nxq"][:], TQ))
                wkc = awts.tile([P, KD, DC], FP8)
                nc.sync.dma_start(wkc, _r(din["wkc"][:], DC))
                nxt = ainp.tile([P, KD, S], FP8)
                nc.sync.dma_start(nxt, _r(din["nxt"][:], S))
                wvc = awts.tile([P, KD, DC], FP8)
                nc.sync.dma_start(wvc, _r(din["wvc"][:], DC))
                xt = ainp.tile([P, KD, TQ], F32)
                nc.sync.dma_start(xt, _r(din["xt"][:], TQ))
                wr = awts.tile([P, KD, E], F32)
                nc.sync.dma_start(wr, _r(din["wr"][:], E))

                ones_f8 = const.tile([P, 1], FP8)
                nc.vector.memset(ones_f8, 1.0)

                qT = aact.tile([P, KD, TQ], FP8)
                kcT = aact.tile([P, 2, S], FP8)
                vc = aact.tile([P, KD, DC], FP8)
                probs = aact.tile([P, KD, H, TQ], FP8)
                rcpd = aact.tile([1, H, TQ], F32)
                rcpb = aact.tile([P, H, TQ], F32)
                oT = aact.tile([P, KD, TQ], FP8)

                # Q^T [d, tq] = Wq @ nxq  (wq loaded in two halves so the
                # first matmuls can start early)
                wq = awts.tile([P, KD, D], FP8)
                nc.sync.dma_start(wq[:, :, :512], _r(din["wq"][:], D)[:, :, :512])
                nc.sync.dma_start(wq[:, :, 512:], _r(din["wq"][:], D)[:, :, 512:])
                for m in range(KD):
                    ps = psA.tile([P, 512], F32, tag="mm")
                    for k in range(0, KD, 2):
                        nc.tensor.matmul(
                            ps[:, :TQ], wq[:, k:k + 2, m * P:(m + 1) * P],
                            nxq[:, k:k + 2, :], start=(k == 0),
                            stop=(k == KD - 2), perf_mode=DR)
                    nc.scalar.activation(qT[:, m, :], ps[:, :TQ], ACTF.Copy,
                                         scale=1.0 / WS)

                # Kc^T [dc, S] = Wkc @ nxt
                for m in range(2):
                    for n4 in range(4):
                        ps = psA.tile([P, 512], F32, tag="mm")
                        for k in range(0, KD, 2):
                            nc.tensor.matmul(
                                ps[:, :256], wkc[:, k:k + 2, m * P:(m + 1) * P],
                                nxt[:, k:k + 2, n4 * 256:(n4 + 1) * 256],
                                start=(k == 0), stop=(k == KD - 2),
                                perf_mode=DR)
                        nc.scalar.activation(
                            kcT[:, m, n4 * 256:(n4 + 1) * 256], ps[:, :256],
                            ACTF.Copy, scale=1.0 / WS)

                # Vc [S, dc] = nxt^T @ Wvc^T  (keys-major)
                for kc in range(KD):
                    ps = psA.tile([P, 512], F32, tag="mm")
                    for k in range(0, KD, 2):
                        nc.tensor.matmul(
                            ps[:, :DC], nxt[:, k:k + 2, kc * P:(kc + 1) * P],
                            wvc[:, k:k + 2, :], start=(k == 0),
                            stop=(k == KD - 2), perf_mode=DR)
                    nc.scalar.activation(vc[:, kc, :], ps[:, :DC], ACTF.Copy,
                                         scale=1.0 / WS)

                # scores^T + exp (keys-major, no max-sub needed); qT/kcT are
                # descaled so psum is the raw QK dot product
                # per head: scores+exp then that head's denominator chain;
                # head h+1's score matmuls hide head h's reciprocal latency
                for h in range(H):
                    for kc in range(KD):
                        ps = psA.tile([P, 512], F32, tag="mm")
                        nc.tensor.matmul(
                            ps[:, :TQ], kcT[:, 0:2, kc * P:(kc + 1) * P],
                            qT[:, 2 * h:2 * h + 2, :], start=True, stop=True,
                            perf_mode=DR)
                        nc.scalar.activation(
                            probs[:, kc, h, :], ps[:, :TQ], ACTF.Exp, scale=SCALE)
                    psd = psD.tile([1, TQ], F32, tag="aux")
                    for kc in range(KD):
                        nc.tensor.matmul(
                            psd, ones_f8[:, 0:1], probs[:, kc, h, :],
                            start=(kc == 0), stop=(kc == KD - 1))
                    nc.vector.reciprocal(rcpd[0:1, h, :], psd)
                    psb = psD.tile([P, TQ], F32, tag="aux")
                    nc.tensor.matmul(psb, ones_row, rcpd[0:1, h, :],
                                     start=True, stop=True)
                    nc.scalar.copy(rcpb[:, h, :], psb)

                # prefetch shared-expert weights now: attention inputs are
                # in flight, MoE phase is ~50us away
                nc.sync.dma_start(w1_0, _r(din["sw1"][:], F))
                nc.sync.dma_start(w3_0, _r(din["sw3"][:], F))
                nc.sync.dma_start(w2_0, _r(din["sw2"][:], D))

                # out_h^T = Vc^T @ probs^T, normalized per token; oT carries
                # a WS scale so its small values stay out of fp8 subnormals
                for h in range(H):
                    for m in range(2):
                        ps = psA.tile([P, 512], F32, tag="mm")
                        for kc in range(0, KD, 2):
                            nc.tensor.matmul(
                                ps[:, :TQ], vc[:, kc:kc + 2, m * P:(m + 1) * P],
                                probs[:, kc:kc + 2, h, :],
                                start=(kc == 0), stop=(kc == KD - 2),
                                perf_mode=DR)
                        nc.vector.scalar_tensor_tensor(
                            out=oT[:, 2 * h + m, :], in0=ps[:, :TQ], scalar=WS,
                            in1=rcpb[:, h, :], op0=ALU.mult, op1=ALU.mult)

                # attn proj + residual: X' = Wo @ O + X  (wo streamed);
                # psum carries WS^2 from wo and oT scales
                # rmsnorm2 squares run on scalar inside the Wo loop so the
                # psq accumulation is ready right after the last Wo matmul
                sq = aact.tile([P, KD, TQ], BF16)
                rs = aact.tile([1, TQ], F32)
                sd = aact.tile([1, TQ], F32)
                rsb = aact.tile([P, TQ], F32)
                rstok = aact.tile([P, 2], F32)
                lg = aact.tile([P, 2, E], F32)
                comb = aact.tile([P, 2, E], F32)
                cT = aact.tile([E, TQ], F32)

                for m in range(KD):
                    wo_m = wostr.tile([P, KD, P], FP8, tag="wo")
                    nc.sync.dma_start(wo_m, _r(din["wo"][:], D)[:, :, m * P:(m + 1) * P])
                    ps = psA.tile([P, 512], F32, tag="mm")
                    for k in range(0, KD, 2):
                        nc.tensor.matmul(
                            ps[:, :TQ], wo_m[:, k:k + 2, :],
                            oT[:, k:k + 2, :], start=(k == 0),
                            stop=(k == KD - 2), perf_mode=DR)
                    nc.vector.scalar_tensor_tensor(
                        out=xpT[:, m, :], in0=ps[:, :TQ],
                        scalar=1.0 / (WS * WS), in1=xt[:, m, :],
                        op0=ALU.mult, op1=ALU.add)
                    nc.scalar.square(sq[:, m, :], xpT[:, m, :])

                # ---------------- rmsnorm2 + router ----------------
                # router logit matmuls are emitted first: they depend only on
                # xpT and wr, so the PE isn't stalled by the rs chain
                psr_t = []
                for t in range(2):
                    psr = psD.tile([P, E], F32, tag="aux")
                    for k in range(KD):
                        nc.tensor.matmul(
                            psr, xpT[:, k, t * P:(t + 1) * P], wr[:, k, :],
                            start=(k == 0), stop=(k == KD - 1))
                    psr_t.append(psr)

                psq = psD.tile([1, TQ], F32, tag="aux")
                for k in range(KD):
                    nc.tensor.matmul(psq, ones_bf[:, 0:1], sq[:, k, :],
                                     start=(k == 0), stop=(k == KD - 1))
                nc.scalar.activation(sd[0:1, :], psq, ACTF.Sqrt,
                                     bias=eps1[0:1, :], scale=1.0 / D)
                nc.vector.reciprocal(rs[0:1, :], sd[0:1, :])
                psb = psD.tile([P, TQ], F32, tag="aux")
                nc.tensor.matmul(psb, ones_row, rs[0:1, :], start=True, stop=True)
                nc.scalar.copy(rsb, psb)
                for m in range(KD):
                    nc.vector.tensor_mul(out=nx2[:, m, :], in0=xpT[:, m, :],
                                         in1=rsb)

                # rs in token-major via PE transpose (rows of rsb^T are const)
                for t in range(2):
                    pst = psA.tile([P, 512], F32, tag="mm")
                    nc.tensor.transpose(pst[:, :P], rsb[:, t * P:(t + 1) * P],
                                        ident)
                    nc.scalar.copy(rstok[:, t:t + 1], pst[:, 0:1])

                # logits (token-major, fp32): psr * rs + bias
                for t in range(2):
                    nc.vector.scalar_tensor_tensor(
                        out=lg[:, t, :], in0=psr_t[t], scalar=rstok[:, t:t + 1],
                        in1=ebias_b, op0=ALU.mult, op1=ALU.add)

                # softmax over experts + top-2 renormalized combine weights
                for t in range(2):
                    lgt = lg[:, t, :]
                    m_s = aact.tile([P, 1], F32, tag="sm", name=f"m_{t}")
                    nc.vector.reduce_max(m_s, lgt, axis=AX)
                    negm = aact.tile([P, 1], F32, tag="sm2", name=f"nm_{t}")
                    nc.vector.tensor_scalar_mul(negm, m_s, -1.0)
                    e_s = aact.tile([P, E], F32, tag="sm3", name=f"e_{t}")
                    den = aact.tile([P, 1], F32, tag="sm4", name=f"d_{t}")
                    nc.scalar.activation(e_s, lgt, ACTF.Exp, bias=negm,
                                         accum_out=den)
                    rcp = aact.tile([P, 1], F32, tag="sm5", name=f"r_{t}")
                    nc.vector.reciprocal(rcp, den)
                    rw = aact.tile([P, E], F32, tag="sm6", name=f"rw_{t}")
                    nc.vector.tensor_scalar_mul(rw, e_s, rcp)
                    m1 = aact.tile([P, 1], F32, tag="sm7", name=f"m1_{t}")
                    nc.vector.reduce_max(m1, rw, axis=AX)
                    mask1 = aact.tile([P, E], F32, tag="sm8", name=f"k1_{t}")
                    nc.vector.tensor_scalar(mask1, rw, m1, None, ALU.is_ge)
                    rw2 = aact.tile([P, E], F32, tag="sm9", name=f"rw2_{t}")
                    nc.vector.scalar_tensor_tensor(
                        out=rw2, in0=mask1, scalar=-10.0, in1=rw,
                        op0=ALU.mult, op1=ALU.add)
                    m2 = aact.tile([P, 1], F32, tag="sm10", name=f"m2_{t}")
                    nc.vector.reduce_max(m2, rw2, axis=AX)
                    masktop = aact.tile([P, E], F32, tag="sm11", name=f"kt_{t}")
                    nc.vector.tensor_scalar(masktop, rw, m2, None, ALU.is_ge)
                    er = aact.tile([P, E], F32, tag="sm12", name=f"er_{t}")
                    nc.scalar.activation(er, rw, ACTF.Exp)
                    erm = aact.tile([P, E], F32, tag="sm13", name=f"em_{t}")
                    nc.vector.tensor_mul(out=erm, in0=er, in1=masktop)
                    den2 = aact.tile([P, 1], F32, tag="sm14", name=f"d2_{t}")
                    nc.vector.reduce_sum(den2, erm, axis=AX)
                    rcp2 = aact.tile([P, 1], F32, tag="sm15", name=f"r2_{t}")
                    nc.vector.reciprocal(rcp2, den2)
                    nc.vector.tensor_scalar_mul(comb[:, t, :], erm, rcp2)

                # combine^T [E, TQ] via PE transpose, then row-broadcast
                for t in range(2):
                    pst = psA.tile([P, 512], F32, tag="mm")
                    nc.tensor.transpose(pst[:E, :P], comb[:, t, :], ident)
                    nc.scalar.copy(cT[:, t * P:(t + 1) * P], pst[:E, :P])
                with tc.tile_pool(name="dbounce", bufs=1, space="DRAM") as dbp:
                    cdram = dbp.tile([1, E * TQ], F32)
                    nc.sync.dma_start(
                        cdram[:].rearrange("o (e t) -> (o e) t", e=E), cT)
                    nc.sync.dma_start(
                        cb[:].rearrange("p e t -> p (e t)"),
                        cdram[:].to_broadcast([P, E * TQ]))

            # ---------------- shared expert + 8 routed experts ----------------
            with (
                tc.tile_pool(name="wmoe", bufs=2) as wmoe,
                tc.tile_pool(name="w2moe", bufs=1) as w2moe,
                tc.tile_pool(name="hact", bufs=1) as hact,
                tc.tile_pool(name="mact", bufs=2) as mact,
                tc.tile_pool(name="psM", bufs=4, space="PSUM") as psM,
                tc.tile_pool(name="psW", bufs=3, space="PSUM") as psW,
            ):
                # phase 1: swiglu intermediates for all 9 units (kept in
                # SBUF, fp8); w2 weights prefetch alongside
                hps = []
                w2s = []
                for u in range(E + 1):
                    if u == 0:
                        w1, w3, w2 = w1_0, w3_0, w2_0
                    else:
                        w1 = wmoe.tile([P, KD, F], FP8, tag="w1")
                        nc.sync.dma_start(w1, _r(din["ew1"][u - 1], F))
                        w3 = wmoe.tile([P, KD, F], FP8, tag="w3")
                        nc.sync.dma_start(w3, _r(din["ew3"][u - 1], F))
                        w2 = w2moe.tile([P, KF, D], FP8, name=f"w2_{u}")
                        nc.sync.dma_start(w2, _r(din["ew2"][u - 1], D))
                    w2s.append(w2)

                    hp = hact.tile([P, KF, TQ], FP8, name=f"hp_{u}")
                    hps.append(hp)
                    for m in range(KF):
                        psa = psM.tile([P, TQ], F32, tag="ab")
                        for k in range(0, KD, 2):
                            nc.tensor.matmul(
                                psa, w1[:, k:k + 2, m * P:(m + 1) * P],
                                nx2[:, k:k + 2, :],
                                start=(k == 0), stop=(k == KD - 2),
                                perf_mode=DR)
                        sa = mact.tile([P, TQ], BF16, tag="sa")
                        nc.scalar.activation(sa, psa, ACTF.Silu, scale=1.0 / WS)
                        psb2 = psM.tile([P, TQ], F32, tag="ab")
                        for k in range(0, KD, 2):
                            nc.tensor.matmul(
                                psb2, w3[:, k:k + 2, m * P:(m + 1) * P],
                                nx2[:, k:k + 2, :],
                                start=(k == 0), stop=(k == KD - 2),
                                perf_mode=DR)
                        if u == 0:
                            nc.vector.scalar_tensor_tensor(
                                out=hp[:, m, :], in0=psb2, scalar=HS / WS,
                                in1=sa, op0=ALU.mult, op1=ALU.mult)
                        else:
                            sacb = mact.tile([P, TQ], BF16, tag="tmp")
                            nc.vector.tensor_mul(out=sacb, in0=sa,
                                                 in1=cb[:, u - 1, :])
                            nc.vector.scalar_tensor_tensor(
                                out=hp[:, m, :], in0=psb2, scalar=HS / WS,
                                in1=sacb, op0=ALU.mult, op1=ALU.mult)

                # phase 2: per output chunk, one contained PSUM group sums
                # all 9 units' W2 contributions (matmul accumulation resets
                # PSUM at zero-region granularity, so a group must fully
                # close before another start touches its bank)
                for m in range(KD):
                    acc = psW.tile([P, TQ], F32, tag="eo")
                    for u in range(E + 1):
                        for k in range(0, KF, 2):
                            nc.tensor.matmul(
                                acc, w2s[u][:, k:k + 2, m * P:(m + 1) * P],
                                hps[u][:, k:k + 2, :],
                                start=(u == 0 and k == 0),
                                stop=(u == E and k == KF - 2),
                                perf_mode=DR)
                    nc.vector.scalar_tensor_tensor(
                        out=outT[:, m, :], in0=acc,
                        scalar=1.0 / (WS * HS), in1=xpT[:, m, :],
                        op0=ALU.mult, op1=ALU.add)
                    nc.sync.dma_start(
                        _r(outt[:], TQ)[:, m, :], outT[:, m, :])

    nc.finalize()
    return nc


def _prep_inputs(inputs):
    bf = ml_dtypes.bfloat16
    f8 = ml_dtypes.float8_e4m3
    X = np.asarray(inputs["X"], np.float32)
    g1 = np.asarray(inputs["g1"], np.float32)
    rs1 = 1.0 / np.sqrt(np.mean(X * X, axis=-1, keepdims=True) + EPS)
    nx = X * rs1 * g1                      # [2, S, D] fp32
    nxT = np.ascontiguousarray(np.transpose(nx, (0, 2, 1))).astype(f8)
    XT = np.ascontiguousarray(np.transpose(X, (0, 2, 1)))

    def pm(a):
        """[C*P, N] -> partition-major [P, C*N]."""
        cp, n = a.shape
        return np.ascontiguousarray(
            a.reshape(cp // P, P, n).swapaxes(0, 1).reshape(P, -1))

    def t2(a):
        return pm(np.ascontiguousarray(np.asarray(a, np.float32).T))

    def t3(a):
        a = np.transpose(np.asarray(a, np.float32), (0, 2, 1))
        return np.stack([pm(np.ascontiguousarray(a[e])) for e in range(E)])

    shared = {
        "wq": (t2(inputs["Wq"]) * WS).astype(f8),
        "wkc": (t2(inputs["Wkc"]) * WS).astype(f8),
        "wvc": (t2(inputs["Wvc"]) * WS).astype(f8),
        "wo": (t2(inputs["Wo"]) * WS).astype(f8),
        "wr": t2(inputs["Wr"]).astype(np.float32),
        "ebias": np.asarray(inputs["expert_bias"],
                            np.float32).reshape(1, E),
        "sw1": (t2(inputs["sW1"]) * WS).astype(f8),
        "sw3": (t2(inputs["sW3"]) * WS).astype(f8),
        "sw2": (t2(inputs["sW2"]) * WS).astype(f8),
        "ew1": (t3(inputs["eW1"]) * WS).astype(f8),
        "ew3": (t3(inputs["eW3"]) * WS).astype(f8),
        "ew2": (t3(inputs["eW2"]) * WS).astype(f8),
    }
    in_maps = []
    for c in range(8):
        b, q0 = c // 4, (c % 4) * TQ
        m = dict(shared)

        def pm(a):
            cp, n = a.shape
            return np.ascontiguousarray(
                a.reshape(cp // P, P, n).swapaxes(0, 1).reshape(P, -1))

        m["nxt"] = pm(nxT[b])
        m["nxq"] = pm(np.ascontiguousarray(nxT[b][:, q0:q0 + TQ]))
        m["xt"] = pm(np.ascontiguousarray(XT[b][:, q0:q0 + TQ]))
        in_maps.append(m)
    return in_maps


def run_on_device(inputs, trace=False):
    if "nc" not in _CACHE:
        _CACHE["nc"] = build_program()
    nc = _CACHE["nc"]
    in_maps = _prep_inputs(inputs)
    res = run_bass_kernel_spmd(nc, in_maps, core_ids=list(range(8)),
                               trace=trace)
    out = np.empty((2, S, D), np.float32)
    for c in range(8):
        b, q0 = c // 4, (c % 4) * TQ
        ot = res.results[c]["outt"].reshape(P, KD, TQ).swapaxes(0, 1)
        out[b, q0:q0 + TQ, :] = ot.reshape(D, TQ).T
    return out, res


def kernel(**inputs):
    out, _ = run_on_device(inputs, trace=False)
    return out



# revision 24
# speedup vs baseline: 1.1541x; 1.1541x over previous
"""DeepSeek layer (MLA attention + shared/routed MoE) on 8 TRN2 NeuronCores.

Data-parallel over tokens: core c handles batch c//4, tokens [(c%4)*256, ...).
Activations live feature-major [feature, token] on device; host pre-transposes
weights (bf16) and precomputes the first rmsnorm (depends only on input X).
Router logits are computed in fp32 so top-2 expert selection matches the
reference; expert matmuls run in bf16.
"""

import numpy as np
import ml_dtypes

import concourse.bass as bass
import concourse.tile as tile
from concourse import bacc, mybir
from concourse.bass_utils import run_bass_kernel_spmd
from concourse.masks import make_identity

BF16 = mybir.dt.bfloat16
F32 = mybir.dt.float32
FP8 = mybir.dt.float8e4
DR = mybir.MatmulPerfMode.DoubleRow
AX = mybir.AxisListType.X
ALU = mybir.AluOpType
ACTF = mybir.ActivationFunctionType

# fp8 scale factors: weights are ~N(0, 0.02) and the FFN intermediate is
# ~0.1-scale; both sit in fp8e4's subnormal range unscaled. Scales are
# folded back out on the PSUM->SBUF path.
WS = 64.0            # weight scale for all fp8 weight matrices
HS = 16.0            # scale for the swiglu intermediate hp

P = 128
D = 1024
KD = D // P          # 8 feature chunks
S = 1024             # keys per batch
TQ = 256             # query tokens per core
H = 4
DC = 256             # compressed kv dim == dk
F = 1024
KF = F // P
E = 8
EPS = 1e-6
SCALE = 1.0 / 16.0   # 1/sqrt(dk)

_CACHE = {}


def _r(ap, n=None):
    """Host-permuted DRAM [P, C*N] -> [P, C, N] view (contiguous)."""
    c = ap.shape[-1]
    n = n if n is not None else c // KD
    return ap.rearrange("p (k n) -> p k n", n=n)


def build_program():
    nc = bacc.Bacc(None)

    def dma_k(dst, src_ap, parts):
        """Issue a [P, K, N] DMA as `parts` k-chunk DMAs so the transfer
        spreads across DMA queues instead of serializing on one."""
        kk = dst.shape[-2]
        step = kk // parts
        for i in range(0, kk, step):
            nc.sync.dma_start(dst[:, i:i + step, :], src_ap[:, i:i + step, :])

    # All tensors are host-permuted to partition-major [P, chunks*N] so each
    # DMA is one contiguous segment per partition (descriptor-rate matters).
    din = {}
    for name, shape, dt in [
        ("nxt", [P, KD * S], FP8),
        ("nxq", [P, KD * TQ], FP8),
        ("xt", [P, KD * TQ], F32),
        ("wq", [P, KD * D], FP8),
        ("wkc", [P, KD * DC], FP8),
        ("wvc", [P, KD * DC], FP8),
        ("wo", [P, KD * D], FP8),
        ("wr", [P, KD * E], F32),
        ("ebias", [1, E], F32),
        ("sw1", [P, KD * F], FP8),
        ("sw3", [P, KD * F], FP8),
        ("sw2", [P, KF * D], FP8),
        ("ew1", [E, P, KD * F], FP8),
        ("ew3", [E, P, KD * F], FP8),
        ("ew2", [E, P, KF * D], FP8),
    ]:
        din[name] = nc.dram_tensor(name, shape, dt, kind="ExternalInput")
    outt = nc.dram_tensor("outt", [P, KD * TQ], F32, kind="ExternalOutput")

    with tile.TileContext(nc) as tc:
        with (
            tc.tile_pool(name="const", bufs=1) as const,
            tc.tile_pool(name="persist", bufs=1) as persist,
        ):
            ones_bf = const.tile([P, 1], BF16)
            nc.vector.memset(ones_bf, 1.0)
            ones_cf = const.tile([P, 1], F32)
            nc.vector.memset(ones_cf, 1.0)
            ones_row = const.tile([1, P], F32)
            nc.vector.memset(ones_row, 1.0)
            eps1 = const.tile([1, 1], F32)
            nc.vector.memset(eps1, EPS)
            ident = const.tile([P, P], F32)
            make_identity(nc, ident)
            ebias_b = const.tile([P, E], F32)
            nc.sync.dma_start(ebias_b, din["ebias"][:].to_broadcast([P, E]))

            xpT = persist.tile([P, KD, TQ], F32)      # X' = X + attn out
            nx2 = persist.tile([P, KD, TQ], FP8)      # rmsnorm2(X') fp8
            cb = persist.tile([P, E, TQ], F32)        # combine weights bcast
            outT = persist.tile([P, KD, TQ], F32)

            # shared-expert weights, prefetched during attention so the MoE
            # phase starts without a DMA stall (DMAs issued after the
            # attention inputs below)
            w1_0 = persist.tile([P, KD, F], FP8)
            w3_0 = persist.tile([P, KD, F], FP8)
            w2_0 = persist.tile([P, KF, D], FP8)

            # ---------------- attention ----------------
            with (
                tc.tile_pool(name="ainp", bufs=1) as ainp,
                tc.tile_pool(name="awts", bufs=1) as awts,
                tc.tile_pool(name="wostr", bufs=2) as wostr,
                tc.tile_pool(name="aact", bufs=1) as aact,
                tc.tile_pool(name="psA", bufs=3, space="PSUM") as psA,
                tc.tile_pool(name="psD", bufs=4, space="PSUM") as psD,
            ):
                # DMA issue order: Q-path operands first so PE starts early
                nxq = ainp.tile([P, KD, TQ], FP8)
                nc.sync.dma_start(nxq, _r(din["nxq"][:], TQ))
                wkc = awts.tile([P, KD, DC], FP8)
                nc.sync.dma_start(wkc, _r(din["wkc"][:], DC))
                nxt = ainp.tile([P, KD, S], FP8)
                nc.sync.dma_start(nxt, _r(din["nxt"][:], S))
                wvc = awts.tile([P, KD, DC], FP8)
                nc.sync.dma_start(wvc, _r(din["wvc"][:], DC))
                xt = ainp.tile([P, KD, TQ], F32)
                nc.sync.dma_start(xt, _r(din["xt"][:], TQ))
                wr = awts.tile([P, KD, E], F32)
                nc.sync.dma_start(wr, _r(din["wr"][:], E))

                ones_f8 = const.tile([P, 1], FP8)
                nc.vector.memset(ones_f8, 1.0)

                qT = aact.tile([P, KD, TQ], FP8)
                kcT = aact.tile([P, 2, S], FP8)
                vc = aact.tile([P, KD, DC], FP8)
                probs = aact.tile([P, KD, H, TQ], FP8)
                rcpd = aact.tile([1, H, TQ], F32)
                rcpb = aact.tile([P, H, TQ], F32)
                oT = aact.tile([P, KD, TQ], FP8)

                # Q^T [d, tq] = Wq @ nxq  (wq loaded in two halves so the
                # first matmuls can start early)
                wq = awts.tile([P, KD, D], FP8)
                nc.sync.dma_start(wq[:, :, :512], _r(din["wq"][:], D)[:, :, :512])
                nc.sync.dma_start(wq[:, :, 512:], _r(din["wq"][:], D)[:, :, 512:])
                for m in range(KD):
                    ps = psA.tile([P, 512], F32, tag="mm")
                    for k in range(0, KD, 2):
                        nc.tensor.matmul(
                            ps[:, :TQ], wq[:, k:k + 2, m * P:(m + 1) * P],
                            nxq[:, k:k + 2, :], start=(k == 0),
                            stop=(k == KD - 2), perf_mode=DR)
                    nc.scalar.activation(qT[:, m, :], ps[:, :TQ], ACTF.Copy,
                                         scale=1.0 / WS)

                # Kc^T [dc, S] = Wkc @ nxt
                for m in range(2):
                    for n4 in range(4):
                        ps = psA.tile([P, 512], F32, tag="mm")
                        for k in range(0, KD, 2):
                            nc.tensor.matmul(
                                ps[:, :256], wkc[:, k:k + 2, m * P:(m + 1) * P],
                                nxt[:, k:k + 2, n4 * 256:(n4 + 1) * 256],
                                start=(k == 0), stop=(k == KD - 2),
                                perf_mode=DR)
                        nc.scalar.activation(
                            kcT[:, m, n4 * 256:(n4 + 1) * 256], ps[:, :256],
                            ACTF.Copy, scale=1.0 / WS)

                # Vc [S, dc] = nxt^T @ Wvc^T  (keys-major)
                for kc in range(KD):
                    ps = psA.tile([P, 512], F32, tag="mm")
                    for k in range(0, KD, 2):
                        nc.tensor.matmul(
                            ps[:, :DC], nxt[:, k:k + 2, kc * P:(kc + 1) * P],
                            wvc[:, k:k + 2, :], start=(k == 0),
                            stop=(k == KD - 2), perf_mode=DR)
                    nc.scalar.activation(vc[:, kc, :], ps[:, :DC], ACTF.Copy,
                                         scale=1.0 / WS)

                # scores^T + exp (keys-major, no max-sub needed); qT/kcT are
                # descaled so psum is the raw QK dot product
                for h in range(H):
                    for kc in range(KD):
                        ps = psA.tile([P, 512], F32, tag="mm")
                        nc.tensor.matmul(
                            ps[:, :TQ], kcT[:, 0:2, kc * P:(kc + 1) * P],
                            qT[:, 2 * h:2 * h + 2, :], start=True, stop=True,
                            perf_mode=DR)
                        nc.scalar.activation(
                            probs[:, kc, h, :], ps[:, :TQ], ACTF.Exp, scale=SCALE)

                # prefetch shared-expert weights now: attention inputs are
                # in flight, MoE phase is ~50us away
                nc.sync.dma_start(w1_0, _r(din["sw1"][:], F))
                nc.sync.dma_start(w3_0, _r(din["sw3"][:], F))
                nc.sync.dma_start(w2_0, _r(din["sw2"][:], D))

                # softmax denominators + reciprocal + broadcast
                for h in range(H):
                    psd = psD.tile([1, TQ], F32, tag="aux")
                    for kc in range(KD):
                        nc.tensor.matmul(
                            psd, ones_f8[:, 0:1], probs[:, kc, h, :],
                            start=(kc == 0), stop=(kc == KD - 1))
                    nc.vector.reciprocal(rcpd[0:1, h, :], psd)
                    psb = psD.tile([P, TQ], F32, tag="aux")
                    nc.tensor.matmul(psb, ones_row, rcpd[0:1, h, :],
                                     start=True, stop=True)
                    nc.scalar.copy(rcpb[:, h, :], psb)

                # out_h^T = Vc^T @ probs^T, normalized per token; oT carries
                # a WS scale so its small values stay out of fp8 subnormals
                for h in range(H):
                    for m in range(2):
                        ps = psA.tile([P, 512], F32, tag="mm")
                        for kc in range(0, KD, 2):
                            nc.tensor.matmul(
                                ps[:, :TQ], vc[:, kc:kc + 2, m * P:(m + 1) * P],
                                probs[:, kc:kc + 2, h, :],
                                start=(kc == 0), stop=(kc == KD - 2),
                                perf_mode=DR)
                        nc.vector.scalar_tensor_tensor(
                            out=oT[:, 2 * h + m, :], in0=ps[:, :TQ], scalar=WS,
                            in1=rcpb[:, h, :], op0=ALU.mult, op1=ALU.mult)

                # attn proj + residual: X' = Wo @ O + X  (wo streamed);
                # psum carries WS^2 from wo and oT scales
                # rmsnorm2 squares run on scalar inside the Wo loop so the
                # psq accumulation is ready right after the last Wo matmul
                sq = aact.tile([P, KD, TQ], BF16)
                rs = aact.tile([1, TQ], F32)
                sd = aact.tile([1, TQ], F32)
                rsb = aact.tile([P, TQ], F32)
                rstok = aact.tile([P, 2], F32)
                lg = aact.tile([P, 2, E], F32)
                comb = aact.tile([P, 2, E], F32)
                cT = aact.tile([E, TQ], F32)

                for m in range(KD):
                    wo_m = wostr.tile([P, KD, P], FP8, tag="wo")
                    nc.sync.dma_start(wo_m, _r(din["wo"][:], D)[:, :, m * P:(m + 1) * P])
                    ps = psA.tile([P, 512], F32, tag="mm")
                    for k in range(0, KD, 2):
                        nc.tensor.matmul(
                            ps[:, :TQ], wo_m[:, k:k + 2, :],
                            oT[:, k:k + 2, :], start=(k == 0),
                            stop=(k == KD - 2), perf_mode=DR)
                    nc.vector.scalar_tensor_tensor(
                        out=xpT[:, m, :], in0=ps[:, :TQ],
                        scalar=1.0 / (WS * WS), in1=xt[:, m, :],
                        op0=ALU.mult, op1=ALU.add)

                # ---------------- rmsnorm2 + router ----------------
                for m in range(KD):
                    nc.scalar.square(sq[:, m, :], xpT[:, m, :])
                psq = psD.tile([1, TQ], F32, tag="aux")
                for k in range(KD):
                    nc.tensor.matmul(psq, ones_bf[:, 0:1], sq[:, k, :],
                                     start=(k == 0), stop=(k == KD - 1))
                nc.scalar.activation(sd[0:1, :], psq, ACTF.Sqrt,
                                     bias=eps1[0:1, :], scale=1.0 / D)
                nc.vector.reciprocal(rs[0:1, :], sd[0:1, :])
                psb = psD.tile([P, TQ], F32, tag="aux")
                nc.tensor.matmul(psb, ones_row, rs[0:1, :], start=True, stop=True)
                nc.scalar.copy(rsb, psb)
                for m in range(KD):
                    nc.vector.tensor_mul(out=nx2[:, m, :], in0=xpT[:, m, :],
                                         in1=rsb)

                # rs in token-major via PE transpose (rows of rsb^T are const)
                for t in range(2):
                    pst = psA.tile([P, 512], F32, tag="mm")
                    nc.tensor.transpose(pst[:, :P], rsb[:, t * P:(t + 1) * P],
                                        ident)
                    nc.scalar.copy(rstok[:, t:t + 1], pst[:, 0:1])

                # logits (token-major, fp32): (X'^T_chunk^T @ Wr^T) * rs + bias
                for t in range(2):
                    psr = psD.tile([P, E], F32, tag="aux")
                    for k in range(KD):
                        nc.tensor.matmul(
                            psr, xpT[:, k, t * P:(t + 1) * P], wr[:, k, :],
                            start=(k == 0), stop=(k == KD - 1))
                    nc.vector.scalar_tensor_tensor(
                        out=lg[:, t, :], in0=psr, scalar=rstok[:, t:t + 1],
                        in1=ebias_b, op0=ALU.mult, op1=ALU.add)

                # softmax over experts + top-2 renormalized combine weights
                for t in range(2):
                    lgt = lg[:, t, :]
                    m_s = aact.tile([P, 1], F32, tag="sm", name=f"m_{t}")
                    nc.vector.reduce_max(m_s, lgt, axis=AX)
                    negm = aact.tile([P, 1], F32, tag="sm2", name=f"nm_{t}")
                    nc.vector.tensor_scalar_mul(negm, m_s, -1.0)
                    e_s = aact.tile([P, E], F32, tag="sm3", name=f"e_{t}")
                    den = aact.tile([P, 1], F32, tag="sm4", name=f"d_{t}")
                    nc.scalar.activation(e_s, lgt, ACTF.Exp, bias=negm,
                                         accum_out=den)
                    rcp = aact.tile([P, 1], F32, tag="sm5", name=f"r_{t}")
                    nc.vector.reciprocal(rcp, den)
                    rw = aact.tile([P, E], F32, tag="sm6", name=f"rw_{t}")
                    nc.vector.tensor_scalar_mul(rw, e_s, rcp)
                    m1 = aact.tile([P, 1], F32, tag="sm7", name=f"m1_{t}")
                    nc.vector.reduce_max(m1, rw, axis=AX)
                    mask1 = aact.tile([P, E], F32, tag="sm8", name=f"k1_{t}")
                    nc.vector.tensor_scalar(mask1, rw, m1, None, ALU.is_ge)
                    rw2 = aact.tile([P, E], F32, tag="sm9", name=f"rw2_{t}")
                    nc.vector.scalar_tensor_tensor(
                        out=rw2, in0=mask1, scalar=-10.0, in1=rw,
                        op0=ALU.mult, op1=ALU.add)
                    m2 = aact.tile([P, 1], F32, tag="sm10", name=f"m2_{t}")
                    nc.vector.reduce_max(m2, rw2, axis=AX)
                    masktop = aact.tile([P, E], F32, tag="sm11", name=f"kt_{t}")
                    nc.vector.tensor_scalar(masktop, rw, m2, None, ALU.is_ge)
                    er = aact.tile([P, E], F32, tag="sm12", name=f"er_{t}")
                    nc.scalar.activation(er, rw, ACTF.Exp)
                    erm = aact.tile([P, E], F32, tag="sm13", name=f"em_{t}")
                    nc.vector.tensor_mul(out=erm, in0=er, in1=masktop)
                    den2 = aact.tile([P, 1], F32, tag="sm14", name=f"d2_{t}")
                    nc.vector.reduce_sum(den2, erm, axis=AX)
                    rcp2 = aact.tile([P, 1], F32, tag="sm15", name=f"r2_{t}")
                    nc.vector.reciprocal(rcp2, den2)
                    nc.vector.tensor_scalar_mul(comb[:, t, :], erm, rcp2)

                # combine^T [E, TQ] via PE transpose, then row-broadcast
                for t in range(2):
                    pst = psA.tile([P, 512], F32, tag="mm")
                    nc.tensor.transpose(pst[:E, :P], comb[:, t, :], ident)
                    nc.scalar.copy(cT[:, t * P:(t + 1) * P], pst[:E, :P])
                with tc.tile_pool(name="dbounce", bufs=1, space="DRAM") as dbp:
                    cdram = dbp.tile([1, E * TQ], F32)
                    nc.sync.dma_start(
                        cdram[:].rearrange("o (e t) -> (o e) t", e=E), cT)
                    nc.sync.dma_start(
                        cb[:].rearrange("p e t -> p (e t)"),
                        cdram[:].to_broadcast([P, E * TQ]))

            # ---------------- shared expert + 8 routed experts ----------------
            with (
                tc.tile_pool(name="wmoe", bufs=2) as wmoe,
                tc.tile_pool(name="w2moe", bufs=1) as w2moe,
                tc.tile_pool(name="hact", bufs=1) as hact,
                tc.tile_pool(name="mact", bufs=2) as mact,
                tc.tile_pool(name="psM", bufs=4, space="PSUM") as psM,
                tc.tile_pool(name="psW", bufs=3, space="PSUM") as psW,
            ):
                # phase 1: swiglu intermediates for all 9 units (kept in
                # SBUF, fp8); w2 weights prefetch alongside
                hps = []
                w2s = []
                for u in range(E + 1):
                    if u == 0:
                        w1, w3, w2 = w1_0, w3_0, w2_0
                    else:
                        w1 = wmoe.tile([P, KD, F], FP8, tag="w1")
                        nc.sync.dma_start(w1, _r(din["ew1"][u - 1], F))
                        w3 = wmoe.tile([P, KD, F], FP8, tag="w3")
                        nc.sync.dma_start(w3, _r(din["ew3"][u - 1], F))
                        w2 = w2moe.tile([P, KF, D], FP8, name=f"w2_{u}")
                        nc.sync.dma_start(w2, _r(din["ew2"][u - 1], D))
                    w2s.append(w2)

                    hp = hact.tile([P, KF, TQ], FP8, name=f"hp_{u}")
                    hps.append(hp)
                    for m in range(KF):
                        psa = psM.tile([P, TQ], F32, tag="ab")
                        for k in range(0, KD, 2):
                            nc.tensor.matmul(
                                psa, w1[:, k:k + 2, m * P:(m + 1) * P],
                                nx2[:, k:k + 2, :],
                                start=(k == 0), stop=(k == KD - 2),
                                perf_mode=DR)
                        sa = mact.tile([P, TQ], BF16, tag="sa")
                        nc.scalar.activation(sa, psa, ACTF.Silu, scale=1.0 / WS)
                        psb2 = psM.tile([P, TQ], F32, tag="ab")
                        for k in range(0, KD, 2):
                            nc.tensor.matmul(
                                psb2, w3[:, k:k + 2, m * P:(m + 1) * P],
                                nx2[:, k:k + 2, :],
                                start=(k == 0), stop=(k == KD - 2),
                                perf_mode=DR)
                        if u == 0:
                            nc.vector.scalar_tensor_tensor(
                                out=hp[:, m, :], in0=psb2, scalar=HS / WS,
                                in1=sa, op0=ALU.mult, op1=ALU.mult)
                        else:
                            sacb = mact.tile([P, TQ], BF16, tag="tmp")
                            nc.vector.tensor_mul(out=sacb, in0=sa,
                                                 in1=cb[:, u - 1, :])
                            nc.vector.scalar_tensor_tensor(
                                out=hp[:, m, :], in0=psb2, scalar=HS / WS,
                                in1=sacb, op0=ALU.mult, op1=ALU.mult)

                # phase 2: per output chunk, one contained PSUM group sums
                # all 9 units' W2 contributions (matmul accumulation resets
                # PSUM at zero-region granularity, so a group must fully
                # close before another start touches its bank)
                for m in range(KD):
                    acc = psW.tile([P, TQ], F32, tag="eo")
                    for u in range(E + 1):
                        for k in range(0, KF, 2):
                            nc.tensor.matmul(
                                acc, w2s[u][:, k:k + 2, m * P:(m + 1) * P],
                                hps[u][:, k:k + 2, :],
                                start=(u == 0 and k == 0),
                                stop=(u == E and k == KF - 2),
                                perf_mode=DR)
                    nc.vector.scalar_tensor_tensor(
                        out=outT[:, m, :], in0=acc,
                        scalar=1.0 / (WS * HS), in1=xpT[:, m, :],
                        op0=ALU.mult, op1=ALU.add)
                    nc.sync.dma_start(
                        _r(outt[:], TQ)[:, m, :], outT[:, m, :])

    nc.finalize()
    return nc


def _prep_inputs(inputs):
    bf = ml_dtypes.bfloat16
    f8 = ml_dtypes.float8_e4m3
    X = np.asarray(inputs["X"], np.float32)
    g1 = np.asarray(inputs["g1"], np.float32)
    rs1 = 1.0 / np.sqrt(np.mean(X * X, axis=-1, keepdims=True) + EPS)
    nx = X * rs1 * g1                      # [2, S, D] fp32
    nxT = np.ascontiguousarray(np.transpose(nx, (0, 2, 1))).astype(f8)
    XT = np.ascontiguousarray(np.transpose(X, (0, 2, 1)))

    def pm(a):
        """[C*P, N] -> partition-major [P, C*N]."""
        cp, n = a.shape
        return np.ascontiguousarray(
            a.reshape(cp // P, P, n).swapaxes(0, 1).reshape(P, -1))

    def t2(a):
        return pm(np.ascontiguousarray(np.asarray(a, np.float32).T))

    def t3(a):
        a = np.transpose(np.asarray(a, np.float32), (0, 2, 1))
        return np.stack([pm(np.ascontiguousarray(a[e])) for e in range(E)])

    shared = {
        "wq": (t2(inputs["Wq"]) * WS).astype(f8),
        "wkc": (t2(inputs["Wkc"]) * WS).astype(f8),
        "wvc": (t2(inputs["Wvc"]) * WS).astype(f8),
        "wo": (t2(inputs["Wo"]) * WS).astype(f8),
        "wr": t2(inputs["Wr"]).astype(np.float32),
        "ebias": np.asarray(inputs["expert_bias"],
                            np.float32).reshape(1, E),
        "sw1": (t2(inputs["sW1"]) * WS).astype(f8),
        "sw3": (t2(inputs["sW3"]) * WS).astype(f8),
        "sw2": (t2(inputs["sW2"]) * WS).astype(f8),
        "ew1": (t3(inputs["eW1"]) * WS).astype(f8),
        "ew3": (t3(inputs["eW3"]) * WS).astype(f8),
        "ew2": (t3(inputs["eW2"]) * WS).astype(f8),
    }
    in_maps = []
    for c in range(8):
        b, q0 = c // 4, (c % 4) * TQ
        m = dict(shared)

        def pm(a):
            cp, n = a.shape
            return np.ascontiguousarray(
                a.reshape(cp // P, P, n).swapaxes(0, 1).reshape(P, -1))

        m["nxt"] = pm(nxT[b])
        m["nxq"] = pm(np.ascontiguousarray(nxT[b][:, q0:q0 + TQ]))
        m["xt"] = pm(np.ascontiguousarray(XT[b][:, q0:q0 + TQ]))
        in_maps.append(m)
    return in_maps


def run_on_device(inputs, trace=False):
    if "nc" not in _CACHE:
        _CACHE["nc"] = build_program()
    nc = _CACHE["nc"]
    in_maps = _prep_inputs(inputs)
    res = run_bass_kernel_spmd(nc, in_maps, core_ids=list(range(8)),
                               trace=trace)
    out = np.empty((2, S, D), np.float32)
    for c in range(8):
        b, q0 = c // 4, (c % 4) * TQ
        ot = res.results[c]["outt"].reshape(P, KD, TQ).swapaxes(0, 1)
        out[b, q0:q0 + TQ, :] = ot.reshape(D, TQ).T
    return out, res


def kernel(**inputs):
    out, _ = run_on_device(inputs, trace=False)
    return out



# revision 27
# speedup vs baseline: 1.1785x; 1.0211x over previous
"""DeepSeek layer (MLA attention + shared/routed MoE) on 8 TRN2 NeuronCores.

Data-parallel over tokens: core c handles batch c//4, tokens [(c%4)*256, ...).
Activations live feature-major [feature, token] on device; host pre-transposes
weights (bf16) and precomputes the first rmsnorm (depends only on input X).
Router logits are computed in fp32 so top-2 expert selection matches the
reference; expert matmuls run in bf16.
"""

import numpy as np
import ml_dtypes

import concourse.bass as bass
import concourse.tile as tile
from concourse import bacc, mybir
from concourse.bass_utils import run_bass_kernel_spmd
from concourse.masks import make_identity

BF16 = mybir.dt.bfloat16
F32 = mybir.dt.float32
FP8 = mybir.dt.float8e4
DR = mybir.MatmulPerfMode.DoubleRow
AX = mybir.AxisListType.X
ALU = mybir.AluOpType
ACTF = mybir.ActivationFunctionType

# fp8 scale factors: weights are ~N(0, 0.02) and the FFN intermediate is
# ~0.1-scale; both sit in fp8e4's subnormal range unscaled. Scales are
# folded back out on the PSUM->SBUF path.
WS = 64.0            # weight scale for all fp8 weight matrices
HS = 16.0            # scale for the swiglu intermediate hp

P = 128
D = 1024
KD = D // P          # 8 feature chunks
S = 1024             # keys per batch
TQ = 256             # query tokens per core
H = 4
DC = 256             # compressed kv dim == dk
F = 1024
KF = F // P
E = 8
EPS = 1e-6
SCALE = 1.0 / 16.0   # 1/sqrt(dk)

_CACHE = {}


def _r(ap, n=None):
    """Host-permuted DRAM [P, C*N] -> [P, C, N] view (contiguous)."""
    c = ap.shape[-1]
    n = n if n is not None else c // KD
    return ap.rearrange("p (k n) -> p k n", n=n)


def build_program():
    nc = bacc.Bacc(None)

    def dma_k(dst, src_ap, parts):
        """Issue a [P, K, N] DMA as `parts` k-chunk DMAs so the transfer
        spreads across DMA queues instead of serializing on one."""
        kk = dst.shape[-2]
        step = kk // parts
        for i in range(0, kk, step):
            nc.sync.dma_start(dst[:, i:i + step, :], src_ap[:, i:i + step, :])

    # All tensors are host-permuted to partition-major [P, chunks*N] so each
    # DMA is one contiguous segment per partition (descriptor-rate matters).
    din = {}
    for name, shape, dt in [
        ("nxt", [P, KD * S], FP8),
        ("nxq", [P, KD * TQ], FP8),
        ("xt", [P, KD * TQ], F32),
        ("wq", [P, KD * D], FP8),
        ("wkc", [P, KD * DC], FP8),
        ("wvc", [P, KD * DC], FP8),
        ("wo", [P, KD * D], FP8),
        ("wr", [P, KD * E], F32),
        ("ebias", [1, E], F32),
        ("sw1", [P, KD * F], FP8),
        ("sw3", [P, KD * F], FP8),
        ("sw2", [P, KF * D], FP8),
        ("ew1", [E, P, KD * F], FP8),
        ("ew3", [E, P, KD * F], FP8),
        ("ew2", [E, P, KF * D], FP8),
    ]:
        din[name] = nc.dram_tensor(name, shape, dt, kind="ExternalInput")
    outt = nc.dram_tensor("outt", [P, KD * TQ], F32, kind="ExternalOutput")

    with tile.TileContext(nc) as tc:
        with (
            tc.tile_pool(name="const", bufs=1) as const,
            tc.tile_pool(name="persist", bufs=1) as persist,
        ):
            ones_bf = const.tile([P, 1], BF16)
            nc.vector.memset(ones_bf, 1.0)
            ones_cf = const.tile([P, 1], F32)
            nc.vector.memset(ones_cf, 1.0)
            ones_row = const.tile([1, P], F32)
            nc.vector.memset(ones_row, 1.0)
            eps1 = const.tile([1, 1], F32)
            nc.vector.memset(eps1, EPS)
            ident = const.tile([P, P], F32)
            make_identity(nc, ident)
            ebias_b = const.tile([P, E], F32)
            nc.sync.dma_start(ebias_b, din["ebias"][:].to_broadcast([P, E]))

            xpT = persist.tile([P, KD, TQ], F32)      # X' = X + attn out
            nx2 = persist.tile([P, KD, TQ], FP8)      # rmsnorm2(X') fp8
            cb = persist.tile([P, E, TQ], F32)        # combine weights bcast
            outT = persist.tile([P, KD, TQ], F32)

            # shared-expert weights, prefetched during attention so the MoE
            # phase starts without a DMA stall (DMAs issued after the
            # attention inputs below)
            w1_0 = persist.tile([P, KD, F], FP8)
            w3_0 = persist.tile([P, KD, F], FP8)
            w2_0 = persist.tile([P, KF, D], FP8)

            # ---------------- attention ----------------
            with (
                tc.tile_pool(name="ainp", bufs=1) as ainp,
                tc.tile_pool(name="awts", bufs=1) as awts,
                tc.tile_pool(name="wostr", bufs=2) as wostr,
                tc.tile_pool(name="aact", bufs=1) as aact,
                tc.tile_pool(name="psA", bufs=3, space="PSUM") as psA,
                tc.tile_pool(name="psD", bufs=4, space="PSUM") as psD,
            ):
                # DMA issue order: Q-path operands first so PE starts early
                nxq = ainp.tile([P, KD, TQ], FP8)
                nc.sync.dma_start(nxq, _r(din["nxq"][:], TQ))
                wkc = awts.tile([P, KD, DC], FP8)
                nc.sync.dma_start(wkc, _r(din["wkc"][:], DC))
                nxt = ainp.tile([P, KD, S], FP8)
                nc.sync.dma_start(nxt, _r(din["nxt"][:], S))
                wvc = awts.tile([P, KD, DC], FP8)
                nc.sync.dma_start(wvc, _r(din["wvc"][:], DC))

                ones_f8 = const.tile([P, 1], FP8)
                nc.vector.memset(ones_f8, 1.0)

                qT = aact.tile([P, KD, TQ], FP8)
                kcT = aact.tile([P, 2, S], FP8)
                vc = aact.tile([P, KD, DC], FP8)
                probs = aact.tile([P, KD, H, TQ], FP8)
                rcpd = aact.tile([1, H, TQ], F32)
                rcpb = aact.tile([P, H, TQ], F32)
                oT = aact.tile([P, KD, TQ], FP8)

                # Q^T [d, tq] = Wq @ nxq  (wq loaded in two halves so the
                # first matmuls can start early)
                wq = awts.tile([P, KD, D], FP8)
                nc.sync.dma_start(wq[:, :, :512], _r(din["wq"][:], D)[:, :, :512])
                nc.sync.dma_start(wq[:, :, 512:], _r(din["wq"][:], D)[:, :, 512:])
                # xt/wr load after the Q-path: xt (1MB f32) is not consumed
                # until the Wo residual add, ~35us after the first matmul
                xt = ainp.tile([P, KD, TQ], F32)
                nc.sync.dma_start(xt, _r(din["xt"][:], TQ))
                wr = awts.tile([P, KD, E], F32)
                nc.sync.dma_start(wr, _r(din["wr"][:], E))
                for m in range(KD):
                    ps = psA.tile([P, 512], F32, tag="mm")
                    for k in range(0, KD, 2):
                        nc.tensor.matmul(
                            ps[:, :TQ], wq[:, k:k + 2, m * P:(m + 1) * P],
                            nxq[:, k:k + 2, :], start=(k == 0),
                            stop=(k == KD - 2), perf_mode=DR)
                    nc.scalar.activation(qT[:, m, :], ps[:, :TQ], ACTF.Copy,
                                         scale=1.0 / WS)

                # Kc^T [dc, S] = Wkc @ nxt
                for m in range(2):
                    for n4 in range(4):
                        ps = psA.tile([P, 512], F32, tag="mm")
                        for k in range(0, KD, 2):
                            nc.tensor.matmul(
                                ps[:, :256], wkc[:, k:k + 2, m * P:(m + 1) * P],
                                nxt[:, k:k + 2, n4 * 256:(n4 + 1) * 256],
                                start=(k == 0), stop=(k == KD - 2),
                                perf_mode=DR)
                        nc.scalar.activation(
                            kcT[:, m, n4 * 256:(n4 + 1) * 256], ps[:, :256],
                            ACTF.Copy, scale=1.0 / WS)

                # Vc [S, dc] = nxt^T @ Wvc^T  (keys-major)
                for kc in range(KD):
                    ps = psA.tile([P, 512], F32, tag="mm")
                    for k in range(0, KD, 2):
                        nc.tensor.matmul(
                            ps[:, :DC], nxt[:, k:k + 2, kc * P:(kc + 1) * P],
                            wvc[:, k:k + 2, :], start=(k == 0),
                            stop=(k == KD - 2), perf_mode=DR)
                    nc.scalar.activation(vc[:, kc, :], ps[:, :DC], ACTF.Copy,
                                         scale=1.0 / WS)

                # scores^T + exp (keys-major, no max-sub needed); qT/kcT are
                # descaled so psum is the raw QK dot product
                for h in range(H):
                    for kc in range(KD):
                        ps = psA.tile([P, 512], F32, tag="mm")
                        nc.tensor.matmul(
                            ps[:, :TQ], kcT[:, 0:2, kc * P:(kc + 1) * P],
                            qT[:, 2 * h:2 * h + 2, :], start=True, stop=True,
                            perf_mode=DR)
                        nc.scalar.activation(
                            probs[:, kc, h, :], ps[:, :TQ], ACTF.Exp, scale=SCALE)

                # prefetch shared-expert weights now: attention inputs are
                # in flight, MoE phase is ~50us away
                nc.sync.dma_start(w1_0, _r(din["sw1"][:], F))
                nc.sync.dma_start(w3_0, _r(din["sw3"][:], F))
                nc.sync.dma_start(w2_0, _r(din["sw2"][:], D))

                # softmax denominators + reciprocal + broadcast
                for h in range(H):
                    psd = psD.tile([1, TQ], F32, tag="aux")
                    for kc in range(KD):
                        nc.tensor.matmul(
                            psd, ones_f8[:, 0:1], probs[:, kc, h, :],
                            start=(kc == 0), stop=(kc == KD - 1))
                    nc.vector.reciprocal(rcpd[0:1, h, :], psd)
                    psb = psD.tile([P, TQ], F32, tag="aux")
                    nc.tensor.matmul(psb, ones_row, rcpd[0:1, h, :],
                                     start=True, stop=True)
                    nc.scalar.copy(rcpb[:, h, :], psb)

                # out_h^T = Vc^T @ probs^T, normalized per token; oT carries
                # a WS scale so its small values stay out of fp8 subnormals
                for h in range(H):
                    for m in range(2):
                        ps = psA.tile([P, 512], F32, tag="mm")
                        for kc in range(0, KD, 2):
                            nc.tensor.matmul(
                                ps[:, :TQ], vc[:, kc:kc + 2, m * P:(m + 1) * P],
                                probs[:, kc:kc + 2, h, :],
                                start=(kc == 0), stop=(kc == KD - 2),
                                perf_mode=DR)
                        nc.vector.scalar_tensor_tensor(
                            out=oT[:, 2 * h + m, :], in0=ps[:, :TQ], scalar=WS,
                            in1=rcpb[:, h, :], op0=ALU.mult, op1=ALU.mult)

                # attn proj + residual: X' = Wo @ O + X  (wo streamed);
                # psum carries WS^2 from wo and oT scales
                # rmsnorm2 squares run on scalar inside the Wo loop so the
                # psq accumulation is ready right after the last Wo matmul
                sq = aact.tile([P, KD, TQ], BF16)
                rs = aact.tile([1, TQ], F32)
                sd = aact.tile([1, TQ], F32)
                rsb = aact.tile([P, TQ], F32)
                rstok = aact.tile([P, 2], F32)
                lg = aact.tile([P, 2, E], F32)
                comb = aact.tile([P, 2, E], F32)
                cT = aact.tile([E, TQ], F32)

                for m in range(KD):
                    wo_m = wostr.tile([P, KD, P], FP8, tag="wo")
                    nc.sync.dma_start(wo_m, _r(din["wo"][:], D)[:, :, m * P:(m + 1) * P])
                    ps = psA.tile([P, 512], F32, tag="mm")
                    for k in range(0, KD, 2):
                        nc.tensor.matmul(
                            ps[:, :TQ], wo_m[:, k:k + 2, :],
                            oT[:, k:k + 2, :], start=(k == 0),
                            stop=(k == KD - 2), perf_mode=DR)
                    nc.vector.scalar_tensor_tensor(
                        out=xpT[:, m, :], in0=ps[:, :TQ],
                        scalar=1.0 / (WS * WS), in1=xt[:, m, :],
                        op0=ALU.mult, op1=ALU.add)

                # ---------------- rmsnorm2 + router ----------------
                for m in range(KD):
                    nc.scalar.square(sq[:, m, :], xpT[:, m, :])
                psq = psD.tile([1, TQ], F32, tag="aux")
                for k in range(KD):
                    nc.tensor.matmul(psq, ones_bf[:, 0:1], sq[:, k, :],
                                     start=(k == 0), stop=(k == KD - 1))
                nc.scalar.activation(sd[0:1, :], psq, ACTF.Sqrt,
                                     bias=eps1[0:1, :], scale=1.0 / D)
                nc.vector.reciprocal(rs[0:1, :], sd[0:1, :])
                psb = psD.tile([P, TQ], F32, tag="aux")
                nc.tensor.matmul(psb, ones_row, rs[0:1, :], start=True, stop=True)
                nc.scalar.copy(rsb, psb)
                for m in range(KD):
                    nc.vector.tensor_mul(out=nx2[:, m, :], in0=xpT[:, m, :],
                                         in1=rsb)

                # rs in token-major via PE transpose (rows of rsb^T are const)
                for t in range(2):
                    pst = psA.tile([P, 512], F32, tag="mm")
                    nc.tensor.transpose(pst[:, :P], rsb[:, t * P:(t + 1) * P],
                                        ident)
                    nc.scalar.copy(rstok[:, t:t + 1], pst[:, 0:1])

                # logits (token-major, fp32): (X'^T_chunk^T @ Wr^T) * rs + bias
                for t in range(2):
                    psr = psD.tile([P, E], F32, tag="aux")
                    for k in range(KD):
                        nc.tensor.matmul(
                            psr, xpT[:, k, t * P:(t + 1) * P], wr[:, k, :],
                            start=(k == 0), stop=(k == KD - 1))
                    nc.vector.scalar_tensor_tensor(
                        out=lg[:, t, :], in0=psr, scalar=rstok[:, t:t + 1],
                        in1=ebias_b, op0=ALU.mult, op1=ALU.add)

                # softmax over experts + top-2 renormalized combine weights
                for t in range(2):
                    lgt = lg[:, t, :]
                    m_s = aact.tile([P, 1], F32, tag="sm", name=f"m_{t}")
                    nc.vector.reduce_max(m_s, lgt, axis=AX)
                    negm = aact.tile([P, 1], F32, tag="sm2", name=f"nm_{t}")
                    nc.vector.tensor_scalar_mul(negm, m_s, -1.0)
                    e_s = aact.tile([P, E], F32, tag="sm3", name=f"e_{t}")
                    den = aact.tile([P, 1], F32, tag="sm4", name=f"d_{t}")
                    nc.scalar.activation(e_s, lgt, ACTF.Exp, bias=negm,
                                         accum_out=den)
                    rcp = aact.tile([P, 1], F32, tag="sm5", name=f"r_{t}")
                    nc.vector.reciprocal(rcp, den)
                    rw = aact.tile([P, E], F32, tag="sm6", name=f"rw_{t}")
                    nc.vector.tensor_scalar_mul(rw, e_s, rcp)
                    m1 = aact.tile([P, 1], F32, tag="sm7", name=f"m1_{t}")
                    nc.vector.reduce_max(m1, rw, axis=AX)
                    mask1 = aact.tile([P, E], F32, tag="sm8", name=f"k1_{t}")
                    nc.vector.tensor_scalar(mask1, rw, m1, None, ALU.is_ge)
                    rw2 = aact.tile([P, E], F32, tag="sm9", name=f"rw2_{t}")
                    nc.vector.scalar_tensor_tensor(
                        out=rw2, in0=mask1, scalar=-10.0, in1=rw,
                        op0=ALU.mult, op1=ALU.add)
                    m2 = aact.tile([P, 1], F32, tag="sm10", name=f"m2_{t}")
                    nc.vector.reduce_max(m2, rw2, axis=AX)
                    masktop = aact.tile([P, E], F32, tag="sm11", name=f"kt_{t}")
                    nc.vector.tensor_scalar(masktop, rw, m2, None, ALU.is_ge)
                    er = aact.tile([P, E], F32, tag="sm12", name=f"er_{t}")
                    nc.scalar.activation(er, rw, ACTF.Exp)
                    erm = aact.tile([P, E], F32, tag="sm13", name=f"em_{t}")
                    nc.vector.tensor_mul(out=erm, in0=er, in1=masktop)
                    den2 = aact.tile([P, 1], F32, tag="sm14", name=f"d2_{t}")
                    nc.vector.reduce_sum(den2, erm, axis=AX)
                    rcp2 = aact.tile([P, 1], F32, tag="sm15", name=f"r2_{t}")
                    nc.vector.reciprocal(rcp2, den2)
                    nc.vector.tensor_scalar_mul(comb[:, t, :], erm, rcp2)

                # combine^T [E, TQ] via PE transpose, then row-broadcast
                for t in range(2):
                    pst = psA.tile([P, 512], F32, tag="mm")
                    nc.tensor.transpose(pst[:E, :P], comb[:, t, :], ident)
                    nc.scalar.copy(cT[:, t * P:(t + 1) * P], pst[:E, :P])
                with tc.tile_pool(name="dbounce", bufs=1, space="DRAM") as dbp:
                    cdram = dbp.tile([1, E * TQ], F32)
                    nc.sync.dma_start(
                        cdram[:].rearrange("o (e t) -> (o e) t", e=E), cT)
                    nc.sync.dma_start(
                        cb[:].rearrange("p e t -> p (e t)"),
                        cdram[:].to_broadcast([P, E * TQ]))

            # ---------------- shared expert + 8 routed experts ----------------
            with (
                tc.tile_pool(name="wmoe", bufs=2) as wmoe,
                tc.tile_pool(name="w2moe", bufs=1) as w2moe,
                tc.tile_pool(name="hact", bufs=1) as hact,
                tc.tile_pool(name="mact", bufs=2) as mact,
                tc.tile_pool(name="psM", bufs=4, space="PSUM") as psM,
                tc.tile_pool(name="psW", bufs=3, space="PSUM") as psW,
            ):
                # phase 1: swiglu intermediates for all 9 units (kept in
                # SBUF, fp8); w2 weights prefetch alongside
                hps = []
                w2s = []
                for u in range(E + 1):
                    if u == 0:
                        w1, w3, w2 = w1_0, w3_0, w2_0
                    else:
                        w1 = wmoe.tile([P, KD, F], FP8, tag="w1")
                        nc.sync.dma_start(w1, _r(din["ew1"][u - 1], F))
                        w3 = wmoe.tile([P, KD, F], FP8, tag="w3")
                        nc.sync.dma_start(w3, _r(din["ew3"][u - 1], F))
                        w2 = w2moe.tile([P, KF, D], FP8, name=f"w2_{u}")
                        nc.sync.dma_start(w2, _r(din["ew2"][u - 1], D))
                    w2s.append(w2)

                    hp = hact.tile([P, KF, TQ], FP8, name=f"hp_{u}")
                    hps.append(hp)
                    for m in range(KF):
                        psa = psM.tile([P, TQ], F32, tag="ab")
                        for k in range(0, KD, 2):
                            nc.tensor.matmul(
                                psa, w1[:, k:k + 2, m * P:(m + 1) * P],
                                nx2[:, k:k + 2, :],
                                start=(k == 0), stop=(k == KD - 2),
                                perf_mode=DR)
                        sa = mact.tile([P, TQ], BF16, tag="sa")
                        nc.scalar.activation(sa, psa, ACTF.Silu, scale=1.0 / WS)
                        psb2 = psM.tile([P, TQ], F32, tag="ab")
                        for k in range(0, KD, 2):
                            nc.tensor.matmul(
                                psb2, w3[:, k:k + 2, m * P:(m + 1) * P],
                                nx2[:, k:k + 2, :],
                                start=(k == 0), stop=(k == KD - 2),
                                perf_mode=DR)
                        if u == 0:
                            nc.vector.scalar_tensor_tensor(
                                out=hp[:, m, :], in0=psb2, scalar=HS / WS,
                                in1=sa, op0=ALU.mult, op1=ALU.mult)
                        else:
                            sacb = mact.tile([P, TQ], BF16, tag="tmp")
                            nc.vector.tensor_mul(out=sacb, in0=sa,
                                                 in1=cb[:, u - 1, :])
                            nc.vector.scalar_tensor_tensor(
                                out=hp[:, m, :], in0=psb2, scalar=HS / WS,
                                in1=sacb, op0=ALU.mult, op1=ALU.mult)

                # phase 2: per output chunk, one contained PSUM group sums
                # all 9 units' W2 contributions (matmul accumulation resets
                # PSUM at zero-region granularity, so a group must fully
                # close before another start touches its bank)
                for m in range(KD):
                    acc = psW.tile([P, TQ], F32, tag="eo")
                    for u in range(E + 1):
                        for k in range(0, KF, 2):
                            nc.tensor.matmul(
                                acc, w2s[u][:, k:k + 2, m * P:(m + 1) * P],
                                hps[u][:, k:k + 2, :],
                                start=(u == 0 and k == 0),
                                stop=(u == E and k == KF - 2),
                                perf_mode=DR)
                    nc.vector.scalar_tensor_tensor(
                        out=outT[:, m, :], in0=acc,
                        scalar=1.0 / (WS * HS), in1=xpT[:, m, :],
                        op0=ALU.mult, op1=ALU.add)
                    nc.sync.dma_start(
                        _r(outt[:], TQ)[:, m, :], outT[:, m, :])

    nc.finalize()
    return nc


def _prep_inputs(inputs):
    bf = ml_dtypes.bfloat16
    f8 = ml_dtypes.float8_e4m3
    X = np.asarray(inputs["X"], np.float32)
    g1 = np.asarray(inputs["g1"], np.float32)
    rs1 = 1.0 / np.sqrt(np.mean(X * X, axis=-1, keepdims=True) + EPS)
    nx = X * rs1 * g1                      # [2, S, D] fp32
    nxT = np.ascontiguousarray(np.transpose(nx, (0, 2, 1))).astype(f8)
    XT = np.ascontiguousarray(np.transpose(X, (0, 2, 1)))

    def pm(a):
        """[C*P, N] -> partition-major [P, C*N]."""
        cp, n = a.shape
        return np.ascontiguousarray(
            a.reshape(cp // P, P, n).swapaxes(0, 1).reshape(P, -1))

    def t2(a):
        return pm(np.ascontiguousarray(np.asarray(a, np.float32).T))

    def t3(a):
        a = np.transpose(np.asarray(a, np.float32), (0, 2, 1))
        return np.stack([pm(np.ascontiguousarray(a[e])) for e in range(E)])

    shared = {
        "wq": (t2(inputs["Wq"]) * WS).astype(f8),
        "wkc": (t2(inputs["Wkc"]) * WS).astype(f8),
        "wvc": (t2(inputs["Wvc"]) * WS).astype(f8),
        "wo": (t2(inputs["Wo"]) * WS).astype(f8),
        "wr": t2(inputs["Wr"]).astype(np.float32),
        "ebias": np.asarray(inputs["expert_bias"],
                            np.float32).reshape(1, E),
        "sw1": (t2(inputs["sW1"]) * WS).astype(f8),
        "sw3": (t2(inputs["sW3"]) * WS).astype(f8),
        "sw2": (t2(inputs["sW2"]) * WS).astype(f8),
        "ew1": (t3(inputs["eW1"]) * WS).astype(f8),
        "ew3": (t3(inputs["eW3"]) * WS).astype(f8),
        "ew2": (t3(inputs["eW2"]) * WS).astype(f8),
    }
    in_maps = []
    for c in range(8):
        b, q0 = c // 4, (c % 4) * TQ
        m = dict(shared)

        def pm(a):
            cp, n = a.shape
            return np.ascontiguousarray(
                a.reshape(cp // P, P, n).swapaxes(0, 1).reshape(P, -1))

        m["nxt"] = pm(nxT[b])
        m["nxq"] = pm(np.ascontiguousarray(nxT[b][:, q0:q0 + TQ]))
        m["xt"] = pm(np.ascontiguousarray(XT[b][:, q0:q0 + TQ]))
        in_maps.append(m)
    return in_maps


def run_on_device(inputs, trace=False):
    if "nc" not in _CACHE:
        _CACHE["nc"] = build_program()
    nc = _CACHE["nc"]
    in_maps = _prep_inputs(inputs)
    res = run_bass_kernel_spmd(nc, in_maps, core_ids=list(range(8)),
                               trace=trace)
    out = np.empty((2, S, D), np.float32)
    for c in range(8):
        b, q0 = c // 4, (c % 4) * TQ
        ot = res.results[c]["outt"].reshape(P, KD, TQ).swapaxes(0, 1)
        out[b, q0:q0 + TQ, :] = ot.reshape(D, TQ).T
    return out, res


def kernel(**inputs):
    out, _ = run_on_device(inputs, trace=False)
    return out

